# revision 10
# baseline (speedup 1.0000x reference)
"""Trainium2 Bass kernel for the N^3 triplet descriptor (gnn_message_passing).

Strategy: the reference's O(N^3) angular sum factorizes exactly via the
Legendre addition theorem into O(N^2) per-pair vector moments:

  P0 term: (sum_j w_j)^2
  P1 term: |sum_j w_j u_j|^2                  (u = unit displacement)
  P2 term: 1.5*|sum_j w_j u_j u_j^T|_F^2 - 0.5*(sum_j w_j)^2

with w_j = fc(r_ij) * r_ij^n.  Each device accumulates 36 pair moments per
central atom (9 radial powers, 9 S1 components, 9+9 symmetric S2
components); the tiny nonlinear combine runs on host after gathering.

Sharding: 8 cores = 2 i-blocks (96 rows on partitions) x 4 j-chunks (48
neighbors on the free axis). Cross-j-chunk partials are summed on host.

Implementation: raw Bass (no Tile framework) to avoid start/tail barrier
overhead. Single ACT table (natural_log_exp_and_others): r and 1/r come
from exp(+-0.5*ln(r^2+eps)) on the otherwise-idle Scalar engine; fc is a
degree-6 polynomial in r^2 on DVE with an exact (r^2 < RC^2) cutoff mask.
"""

import numpy as np

import concourse.bass as bass
import concourse.bacc as bacc
from concourse import mybir
from concourse.bass_utils import run_bass_kernel_spmd

F32 = mybir.dt.float32
ALU = mybir.AluOpType
ACT = mybir.ActivationFunctionType

N = 192
NI = 96          # i rows per core (partition dim)
NJ = 48          # j neighbors per core (free dim)
NIB = 2          # i blocks
NJC = 4          # j chunks
BOX_L = 20.0
RC = 5.0

# fc(w) = 0.5*(1+cos(pi*sqrt(w)/RC)) as degree-6 poly in w = r^2, w in [0, RC^2]
# (chebyshev fit, max abs err 1.3e-8)
_FC_W = np.linspace(0, RC * RC, 20001)
_FC_Y = 0.5 * (1 + np.cos(np.pi * np.sqrt(_FC_W) / RC))
_FC_C = (
    np.polynomial.chebyshev.Chebyshev.fit(_FC_W, _FC_Y, 6, domain=[0, RC * RC])
    .convert(kind=np.polynomial.Polynomial)
    .coef.astype(np.float64)
)

_cached = {}


def _v(ap, off, dims):
    """Custom free-dim view of an SBUF tile AP: keep partition dim, replace
    free dims, shift offset by `off` elements."""
    return bass.AP(ap.tensor, ap.offset + off, [list(ap.ap[0])] + [list(d) for d in dims])


def build_nc():
    nc = bacc.Bacc(
        "TRN2",
        target_bir_lowering=False,
        debug=False,
        enable_asserts=True,
        num_devices=NIB * NJC,
    )
    rji_d = nc.dram_tensor("rji", [NI, 160], F32, kind="ExternalInput").ap()
    out_d = nc.dram_tensor("out", [NI, 36], F32, kind="ExternalOutput").ap()

    rji = nc.alloc_sbuf_tensor("rji_s", [NI, 160], F32).ap()
    dxr = nc.alloc_sbuf_tensor("dxr", [NI, 144], F32).ap()
    dx = nc.alloc_sbuf_tensor("dx", [NI, 144], F32).ap()
    sq_t = nc.alloc_sbuf_tensor("sq_t", [NI, 144], F32).ap()
    r2 = nc.alloc_sbuf_tensor("r2", [NI, NJ], F32).ap()
    r = nc.alloc_sbuf_tensor("r", [NI, NJ], F32).ap()
    rinv = nc.alloc_sbuf_tensor("rinv", [NI, NJ], F32).ap()
    ln1 = nc.alloc_sbuf_tensor("ln1", [NI, NJ], F32).ap()
    m25 = nc.alloc_sbuf_tensor("m25", [NI, NJ], F32).ap()
    yh = nc.alloc_sbuf_tensor("yh", [NI, NJ], F32).ap()
    yh144 = nc.alloc_sbuf_tensor("yh144", [NI, 144], F32).ap()
    r4 = nc.alloc_sbuf_tensor("r4", [NI, NJ], F32).ap()
    poff = nc.alloc_sbuf_tensor("poff", [NI, 144], F32).ap()
    fcp = nc.alloc_sbuf_tensor("fcp", [NI, 9 * NJ], F32).ap()
    w1 = nc.alloc_sbuf_tensor("w1", [NI, 144], F32).ap()
    w2 = nc.alloc_sbuf_tensor("w2", [NI, 144], F32).ap()
    tt = nc.alloc_sbuf_tensor("tt", [NI, 432], F32).ap()
    bigd = nc.alloc_sbuf_tensor("bigd", [NI, 432], F32).ap()
    bigo = nc.alloc_sbuf_tensor("bigo", [NI, 432], F32).ap()
    sg = nc.alloc_sbuf_tensor("sg", [NI, 36], F32).ap()
    scr = nc.alloc_sbuf_tensor("scr", [1, 8], F32).ap()

    dsem = nc.alloc_semaphore("dsem")
    vq = nc.alloc_semaphore("vq")      # DVE instruction counter
    sqm = nc.alloc_semaphore("sqm")    # ACT instruction counter

    rj3 = rji[:, 0:144].rearrange("p (d j) -> p d j", d=3)
    ri3 = rji[:, 144:147].unsqueeze(-1).broadcast_to((NI, 3, NJ))
    dxr3 = dxr.rearrange("p (d j) -> p d j", d=3)
    dx3 = dx.rearrange("p (d j) -> p d j", d=3)
    rinv3 = rinv.unsqueeze(1).broadcast_to((NI, 3, NJ))

    c = [float(x) for x in _FC_C]

    # DVE instruction indices (vq value after each) for cross-engine waits
    VQ_R2 = 8      # r2 (+eps) ready
    VQ_ALL = 32    # sg complete
    SQ_RINV = 4    # r, rinv ready

    with nc.Block() as block:

        @block.sync
        def _(sync):
            sync.dma_start(rji, rji_d).then_inc(dsem, 16)
            sync.wait_ge(vq, VQ_ALL)
            sync.dma_start(out_d, sg).then_inc(dsem, 16)
            sync.wait_ge(dsem, 32)

        @block.scalar
        def _(scalar):
            sn = [0]

            def S(inst):
                # same-engine ordering chain (TRN2 engines pipeline;
                # RAW hazards need explicit sems — free at runtime)
                if sn[0] > 0:
                    inst._wait_ge(sqm, sn[0])
                inst.then_inc(sqm, 1)
                sn[0] += 1
                return inst

            # dummy exp on an init-time const tile: pulls the single ACT
            # table load to t=0, overlapping input DMA + DVE distance math
            S(scalar.activation(scr[0:1, 0:1], nc.const_aps.tensor(0.0, (1, 1)), ACT.Exp))
            scalar.wait_ge(vq, VQ_R2)
            S(scalar.activation(ln1, r2, ACT.Ln))
            S(scalar.activation(r, ln1, ACT.Exp, scale=0.5))
            S(scalar.activation(rinv, ln1, ACT.Exp, scale=-0.5))
            assert sn[0] == SQ_RINV

        @block.vector
        def _(vector):
            vn = [0]

            def V(inst):
                if vn[0] > 0:
                    inst._wait_ge(vq, vn[0])
                inst.then_inc(vq, 1)
                vn[0] += 1
                return inst

            vector.wait_ge(dsem, 16)
            V(vector.tensor_tensor(dxr3, rj3, ri3, op=ALU.subtract))
            # minimum image (box = BOX_L * I): dx -= L*(dxr>L/2); dx += L*(dxr<-L/2)
            V(vector.tensor_scalar(
                yh144, dxr, BOX_L / 2, BOX_L, op0=ALU.is_gt, op1=ALU.mult))
            V(vector.tensor_tensor(dx, dxr, yh144, op=ALU.subtract))
            V(vector.tensor_scalar(
                yh144, dxr, -BOX_L / 2, BOX_L, op0=ALU.is_lt, op1=ALU.mult))
            V(vector.tensor_tensor(dx, dx, yh144, op=ALU.add))
            V(vector.tensor_tensor(sq_t, dx, dx, op=ALU.mult))
            V(vector.reduce_sum(
                r2, sq_t.rearrange("p (d j) -> p j d", d=3),
                axis=mybir.AxisListType.X,
            ))
            # eps so ln/exp give finite 1/r (u_ii ends up exactly 0)
            V(vector.tensor_scalar(r2, r2, 1e-12, None, op0=ALU.add))
            assert vn[0] == VQ_R2
            # fc = poly6(r2) * (r2 < RC^2), Horner on DVE
            V(vector.tensor_scalar(m25, r2, RC * RC, None, op0=ALU.is_lt))
            V(vector.tensor_scalar(yh, r2, c[6], None, op0=ALU.mult))
            for k in (5, 4, 3, 2, 1):
                V(vector.scalar_tensor_tensor(
                    yh, yh, c[k], r2, op0=ALU.add, op1=ALU.mult))
            fc = fcp[:, 0:NJ]
            V(vector.scalar_tensor_tensor(
                fc, yh, c[0], m25, op0=ALU.add, op1=ALU.mult))
            # fc-independent geometry
            V(vector.tensor_tensor(r4, r2, r2, op=ALU.mult))
            V(vector.tensor_tensor(
                poff[:, 0:96], dx[:, 0:96], dx[:, 48:144], op=ALU.mult))
            V(vector.tensor_tensor(
                poff[:, 96:144], dx[:, 0:NJ], dx[:, 96:144], op=ALU.mult))
            # wait for r, rinv from ACT
            vector.wait_ge(sqm, SQ_RINV)
            # fcp[k] = fc * r^k: evens via r2/r4, odds in one strided mult
            V(vector.tensor_tensor(
                fcp[:, 2 * NJ:3 * NJ], fc, r2, op=ALU.mult))
            V(vector.tensor_tensor(
                _v(fcp, 4 * NJ, [[2 * NJ, 2], [1, NJ]]),
                _v(fcp, 0, [[2 * NJ, 2], [1, NJ]]),
                _v(r4, 0, [[0, 2], [1, NJ]]),
                op=ALU.mult,
            ))
            V(vector.tensor_tensor(
                fcp[:, 8 * NJ:9 * NJ], fcp[:, 4 * NJ:5 * NJ], r4, op=ALU.mult))
            V(vector.tensor_tensor(
                _v(fcp, NJ, [[2 * NJ, 4], [1, NJ]]),
                _v(fcp, 0, [[2 * NJ, 4], [1, NJ]]),
                _v(r, 0, [[0, 4], [1, NJ]]),
                op=ALU.mult,
            ))
            V(vector.reduce_sum(
                sg[:, 0:9], fcp.rearrange("p (k j) -> p k j", k=9),
                axis=mybir.AxisListType.X,
            ))
            # weights: w1_n = fc r^n / r, w2_n = fc r^n / r^2
            V(vector.tensor_tensor(
                w1.rearrange("p (n j) -> p n j", n=3),
                fcp[:, 0:144].rearrange("p (n j) -> p n j", n=3),
                rinv3, op=ALU.mult))
            V(vector.tensor_tensor(
                w2.rearrange("p (n j) -> p n j", n=3),
                w1.rearrange("p (n j) -> p n j", n=3),
                rinv3, op=ALU.mult))
            # S1: T[n,d] = w1_n * dx_d
            V(vector.tensor_tensor(
                tt.rearrange("p (n d j) -> p n d j", n=3, d=3),
                w1.rearrange("p (n j) -> p n j", n=3).unsqueeze(2).broadcast_to((NI, 3, 3, NJ)),
                dx3.unsqueeze(1).broadcast_to((NI, 3, 3, NJ)),
                op=ALU.mult))
            V(vector.reduce_sum(
                sg[:, 9:18], tt.rearrange("p (m j) -> p m j", m=9),
                axis=mybir.AxisListType.X,
            ))
            # S2 diag: w2_n * dx_d^2 (sq_t);  S2 off: w2_n * (xy, yz, xz)
            V(vector.tensor_tensor(
                bigd.rearrange("p (n d j) -> p n d j", n=3, d=3),
                w2.rearrange("p (n j) -> p n j", n=3).unsqueeze(2).broadcast_to((NI, 3, 3, NJ)),
                sq_t.rearrange("p (d j) -> p d j", d=3).unsqueeze(1).broadcast_to((NI, 3, 3, NJ)),
                op=ALU.mult))
            V(vector.reduce_sum(
                sg[:, 18:27], bigd.rearrange("p (m j) -> p m j", m=9),
                axis=mybir.AxisListType.X,
            ))
            V(vector.tensor_tensor(
                bigo.rearrange("p (n m j) -> p n m j", n=3, m=3),
                w2.rearrange("p (n j) -> p n j", n=3).unsqueeze(2).broadcast_to((NI, 3, 3, NJ)),
                poff.rearrange("p (m j) -> p m j", m=3).unsqueeze(1).broadcast_to((NI, 3, 3, NJ)),
                op=ALU.mult))
            V(vector.reduce_sum(
                sg[:, 27:36], bigo.rearrange("p (m j) -> p m j", m=9),
                axis=mybir.AxisListType.X,
            ))
            assert vn[0] == VQ_ALL, vn[0]

    nc.compile()
    return nc


def host_prep(R):
    """Per-core input arrays: [96, 160] = [RjT replicated | Ri | pad]."""
    R = np.ascontiguousarray(R, np.float32)
    in_maps = []
    for core in range(NIB * NJC):
        ib, jc = divmod(core, NJC)
        rji = np.zeros((NI, 160), np.float32)
        rj = R[jc * NJ:(jc + 1) * NJ, :]              # [48, 3]
        rji[:, 0:144] = rj.T.reshape(1, 144)          # d-major, replicated
        rji[:, 144:147] = R[ib * NI:(ib + 1) * NI, :]
        in_maps.append({"rji": rji})
    return in_maps


def host_combine(partials):
    """partials: list of 8 [96,36] arrays (core order). Returns [192,18]."""
    sums = np.zeros((N, 36), np.float64)
    for core, p in enumerate(partials):
        ib = core // NJC
        sums[ib * NI:(ib + 1) * NI] += p.astype(np.float64)
    sums = sums.astype(np.float32)
    q_r = sums[:, 0:9].copy()
    q_r[:, 0] -= 1.0                                  # remove j==i self term
    s0 = q_r[:, 0:3]                                  # [N,3] n=0..2
    s1 = sums[:, 9:18].reshape(N, 3, 3)               # [N,n,d]
    s2d = sums[:, 18:27].reshape(N, 3, 3)             # [N,n,d] diagonal
    s2o = sums[:, 27:36].reshape(N, 3, 3)             # [N,n,m] off-diagonal
    ang = np.empty((N, 3, 3), np.float32)
    ang[:, :, 0] = s0 * s0
    ang[:, :, 1] = (s1 * s1).sum(-1)
    fro2 = (s2d * s2d).sum(-1) + 2.0 * (s2o * s2o).sum(-1)
    ang[:, :, 2] = 1.5 * fro2 - 0.5 * s0 * s0
    return np.concatenate([q_r, ang.reshape(N, 9)], axis=-1)


def _get_nc():
    if "nc" not in _cached:
        _cached["nc"] = build_nc()
    return _cached["nc"]


def kernel(R, box):
    R = np.asarray(R, np.float32)
    box = np.asarray(box, np.float32)
    assert R.shape == (N, 3)
    assert np.allclose(box, np.eye(3, dtype=np.float32) * BOX_L), (
        "kernel compiled for box = 20*I"
    )
    nc = _get_nc()
    in_maps = host_prep(R)
    res = run_bass_kernel_spmd(nc, in_maps, list(range(NIB * NJC)))
    partials = [res.results[c]["out"] for c in range(NIB * NJC)]
    return host_combine(partials)


# revision 12
# speedup vs baseline: 1.0611x; 1.0611x over previous
"""Trainium2 Bass kernel for the N^3 triplet descriptor (gnn_message_passing).

Strategy: the reference's O(N^3) angular sum factorizes exactly via the
Legendre addition theorem into O(N^2) per-pair vector moments:

  P0 term: (sum_j w_j)^2
  P1 term: |sum_j w_j u_j|^2                  (u = unit displacement)
  P2 term: 1.5*|sum_j w_j u_j u_j^T|_F^2 - 0.5*(sum_j w_j)^2

with w_j = fc(r_ij) * r_ij^n.  Each device accumulates 36 pair moments per
central atom (9 radial powers, 9 S1 components, 9+9 symmetric S2
components); the tiny nonlinear combine runs on host after gathering.

Sharding: 8 cores = 2 i-blocks (96 rows on partitions) x 4 j-chunks (48
neighbors on the free axis). Cross-j-chunk partials are summed on host.

Implementation: raw Bass (no Tile framework) to avoid start/tail barrier
overhead. Single ACT table (natural_log_exp_and_others): r and 1/r come
from exp(+-0.5*ln(r^2+eps)) on the otherwise-idle Scalar engine; fc is a
degree-6 polynomial in r^2 on DVE with an exact (r^2 < RC^2) cutoff mask.
"""

import numpy as np

import concourse.bass as bass
import concourse.bacc as bacc
from concourse import mybir
from concourse.bass_utils import run_bass_kernel_spmd

F32 = mybir.dt.float32
ALU = mybir.AluOpType
ACT = mybir.ActivationFunctionType

N = 192
NI = 96          # i rows per core (partition dim)
NJ = 48          # j neighbors per core (free dim)
NIB = 2          # i blocks
NJC = 4          # j chunks
BOX_L = 20.0
RC = 5.0

# fc(w) = 0.5*(1+cos(pi*sqrt(w)/RC)) as degree-6 poly in w = r^2, w in [0, RC^2]
# (chebyshev fit, max abs err 1.3e-8)
_FC_W = np.linspace(0, RC * RC, 20001)
_FC_Y = 0.5 * (1 + np.cos(np.pi * np.sqrt(_FC_W) / RC))
_FC_C = (
    np.polynomial.chebyshev.Chebyshev.fit(_FC_W, _FC_Y, 6, domain=[0, RC * RC])
    .convert(kind=np.polynomial.Polynomial)
    .coef.astype(np.float64)
)

_cached = {}


def _v(ap, off, dims):
    """Custom free-dim view of an SBUF tile AP: keep partition dim, replace
    free dims, shift offset by `off` elements."""
    return bass.AP(ap.tensor, ap.offset + off, [list(ap.ap[0])] + [list(d) for d in dims])


def build_nc():
    nc = bacc.Bacc(
        "TRN2",
        target_bir_lowering=False,
        debug=False,
        enable_asserts=True,
        num_devices=NIB * NJC,
    )
    rji_d = nc.dram_tensor("rji", [NI, 160], F32, kind="ExternalInput").ap()
    out_d = nc.dram_tensor("out", [NI, 36], F32, kind="ExternalOutput").ap()

    rji = nc.alloc_sbuf_tensor("rji_s", [NI, 160], F32).ap()
    dxr = nc.alloc_sbuf_tensor("dxr", [NI, 144], F32).ap()
    dx = nc.alloc_sbuf_tensor("dx", [NI, 144], F32).ap()
    sq_t = nc.alloc_sbuf_tensor("sq_t", [NI, 144], F32).ap()
    r2 = nc.alloc_sbuf_tensor("r2", [NI, NJ], F32).ap()
    r = nc.alloc_sbuf_tensor("r", [NI, NJ], F32).ap()
    rinv = nc.alloc_sbuf_tensor("rinv", [NI, NJ], F32).ap()
    ln1 = nc.alloc_sbuf_tensor("ln1", [NI, NJ], F32).ap()
    m25 = nc.alloc_sbuf_tensor("m25", [NI, NJ], F32).ap()
    yh = nc.alloc_sbuf_tensor("yh", [NI, NJ], F32).ap()
    yh144 = nc.alloc_sbuf_tensor("yh144", [NI, 144], F32).ap()
    r4 = nc.alloc_sbuf_tensor("r4", [NI, NJ], F32).ap()
    poff = nc.alloc_sbuf_tensor("poff", [NI, 144], F32).ap()
    fcp = nc.alloc_sbuf_tensor("fcp", [NI, 9 * NJ], F32).ap()
    w1 = nc.alloc_sbuf_tensor("w1", [NI, 144], F32).ap()
    w2 = nc.alloc_sbuf_tensor("w2", [NI, 144], F32).ap()
    tt = nc.alloc_sbuf_tensor("tt", [NI, 432], F32).ap()
    bigd = nc.alloc_sbuf_tensor("bigd", [NI, 432], F32).ap()
    bigo = nc.alloc_sbuf_tensor("bigo", [NI, 432], F32).ap()
    sg = nc.alloc_sbuf_tensor("sg", [NI, 36], F32).ap()
    scr = nc.alloc_sbuf_tensor("scr", [1, 8], F32).ap()

    dsem = nc.alloc_semaphore("dsem")
    vq = nc.alloc_semaphore("vq")      # DVE instruction counter
    sqm = nc.alloc_semaphore("sqm")    # ACT instruction counter

    rj3 = rji[:, 0:144].rearrange("p (d j) -> p d j", d=3)
    ri3 = rji[:, 144:147].unsqueeze(-1).broadcast_to((NI, 3, NJ))
    dxr3 = dxr.rearrange("p (d j) -> p d j", d=3)
    dx3 = dx.rearrange("p (d j) -> p d j", d=3)
    rinv3 = rinv.unsqueeze(1).broadcast_to((NI, 3, NJ))

    c = [float(x) for x in _FC_C]

    # DVE instruction indices (vq value after each) for cross-engine waits
    VQ_R2 = 8      # r2 (+eps) ready
    VQ_ALL = 32    # sg complete
    SQ_RINV = 4    # r, rinv ready

    with nc.Block() as block:

        @block.sync
        def _(sync):
            sync.dma_start(rji, rji_d).then_inc(dsem, 16)
            sync.wait_ge(vq, VQ_ALL)
            sync.dma_start(out_d, sg).then_inc(dsem, 16)
            sync.wait_ge(dsem, 32)

        @block.scalar
        def _(scalar):
            sn = [0]

            def S(inst):
                # same-engine ordering chain (TRN2 engines pipeline;
                # RAW hazards need explicit sems — free at runtime)
                if sn[0] > 0:
                    inst._wait_ge(sqm, sn[0])
                inst.then_inc(sqm, 1)
                sn[0] += 1
                return inst

            # explicit table load (natural_log_exp_and_others: Ln AND Exp)
            # at t=0, overlapping input DMA + DVE distance math; without it
            # the auto-placement pass thrashes through 3 greedy set loads
            from concourse.hw_specs import get_activation_tables
            set_id = list(get_activation_tables(nc.m.arch)).index(
                "natural_log_exp_and_others")
            S(scalar.add_instruction(mybir.InstLoadActFuncSet(
                name=nc.get_next_instruction_name(),
                act_func_set_id=set_id, ins=[], outs=[])))
            scalar.wait_ge(vq, VQ_R2)
            S(scalar.activation(ln1, r2, ACT.Ln))
            S(scalar.activation(r, ln1, ACT.Exp, scale=0.5))
            S(scalar.activation(rinv, ln1, ACT.Exp, scale=-0.5))
            assert sn[0] == SQ_RINV  # includes the table-load inst

        @block.vector
        def _(vector):
            vn = [0]

            def V(inst):
                if vn[0] > 0:
                    inst._wait_ge(vq, vn[0])
                inst.then_inc(vq, 1)
                vn[0] += 1
                return inst

            vector.wait_ge(dsem, 16)
            V(vector.tensor_tensor(dxr3, rj3, ri3, op=ALU.subtract))
            # minimum image (box = BOX_L * I): dx -= L*(dxr>L/2); dx += L*(dxr<-L/2)
            V(vector.tensor_scalar(
                yh144, dxr, BOX_L / 2, BOX_L, op0=ALU.is_gt, op1=ALU.mult))
            V(vector.tensor_tensor(dx, dxr, yh144, op=ALU.subtract))
            V(vector.tensor_scalar(
                yh144, dxr, -BOX_L / 2, BOX_L, op0=ALU.is_lt, op1=ALU.mult))
            V(vector.tensor_tensor(dx, dx, yh144, op=ALU.add))
            V(vector.tensor_tensor(sq_t, dx, dx, op=ALU.mult))
            V(vector.reduce_sum(
                r2, sq_t.rearrange("p (d j) -> p j d", d=3),
                axis=mybir.AxisListType.X,
            ))
            # eps so ln/exp give finite 1/r (u_ii ends up exactly 0)
            V(vector.tensor_scalar(r2, r2, 1e-12, None, op0=ALU.add))
            assert vn[0] == VQ_R2
            # fc = poly6(r2) * (r2 < RC^2), Horner on DVE
            V(vector.tensor_scalar(m25, r2, RC * RC, None, op0=ALU.is_lt))
            V(vector.tensor_scalar(yh, r2, c[6], None, op0=ALU.mult))
            for k in (5, 4, 3, 2, 1):
                V(vector.scalar_tensor_tensor(
                    yh, yh, c[k], r2, op0=ALU.add, op1=ALU.mult))
            fc = fcp[:, 0:NJ]
            V(vector.scalar_tensor_tensor(
                fc, yh, c[0], m25, op0=ALU.add, op1=ALU.mult))
            # fc-independent geometry
            V(vector.tensor_tensor(r4, r2, r2, op=ALU.mult))
            V(vector.tensor_tensor(
                poff[:, 0:96], dx[:, 0:96], dx[:, 48:144], op=ALU.mult))
            V(vector.tensor_tensor(
                poff[:, 96:144], dx[:, 0:NJ], dx[:, 96:144], op=ALU.mult))
            # wait for r, rinv from ACT
            vector.wait_ge(sqm, SQ_RINV)
            # fcp[k] = fc * r^k: evens via r2/r4, odds in one strided mult
            V(vector.tensor_tensor(
                fcp[:, 2 * NJ:3 * NJ], fc, r2, op=ALU.mult))
            V(vector.tensor_tensor(
                _v(fcp, 4 * NJ, [[2 * NJ, 2], [1, NJ]]),
                _v(fcp, 0, [[2 * NJ, 2], [1, NJ]]),
                _v(r4, 0, [[0, 2], [1, NJ]]),
                op=ALU.mult,
            ))
            V(vector.tensor_tensor(
                fcp[:, 8 * NJ:9 * NJ], fcp[:, 4 * NJ:5 * NJ], r4, op=ALU.mult))
            V(vector.tensor_tensor(
                _v(fcp, NJ, [[2 * NJ, 4], [1, NJ]]),
                _v(fcp, 0, [[2 * NJ, 4], [1, NJ]]),
                _v(r, 0, [[0, 4], [1, NJ]]),
                op=ALU.mult,
            ))
            V(vector.reduce_sum(
                sg[:, 0:9], fcp.rearrange("p (k j) -> p k j", k=9),
                axis=mybir.AxisListType.X,
            ))
            # weights: w1_n = fc r^n / r, w2_n = fc r^n / r^2
            V(vector.tensor_tensor(
                w1.rearrange("p (n j) -> p n j", n=3),
                fcp[:, 0:144].rearrange("p (n j) -> p n j", n=3),
                rinv3, op=ALU.mult))
            V(vector.tensor_tensor(
                w2.rearrange("p (n j) -> p n j", n=3),
                w1.rearrange("p (n j) -> p n j", n=3),
                rinv3, op=ALU.mult))
            # S1: T[n,d] = w1_n * dx_d
            V(vector.tensor_tensor(
                tt.rearrange("p (n d j) -> p n d j", n=3, d=3),
                w1.rearrange("p (n j) -> p n j", n=3).unsqueeze(2).broadcast_to((NI, 3, 3, NJ)),
                dx3.unsqueeze(1).broadcast_to((NI, 3, 3, NJ)),
                op=ALU.mult))
            V(vector.reduce_sum(
                sg[:, 9:18], tt.rearrange("p (m j) -> p m j", m=9),
                axis=mybir.AxisListType.X,
            ))
            # S2 diag: w2_n * dx_d^2 (sq_t);  S2 off: w2_n * (xy, yz, xz)
            V(vector.tensor_tensor(
                bigd.rearrange("p (n d j) -> p n d j", n=3, d=3),
                w2.rearrange("p (n j) -> p n j", n=3).unsqueeze(2).broadcast_to((NI, 3, 3, NJ)),
                sq_t.rearrange("p (d j) -> p d j", d=3).unsqueeze(1).broadcast_to((NI, 3, 3, NJ)),
                op=ALU.mult))
            V(vector.reduce_sum(
                sg[:, 18:27], bigd.rearrange("p (m j) -> p m j", m=9),
                axis=mybir.AxisListType.X,
            ))
            V(vector.tensor_tensor(
                bigo.rearrange("p (n m j) -> p n m j", n=3, m=3),
                w2.rearrange("p (n j) -> p n j", n=3).unsqueeze(2).broadcast_to((NI, 3, 3, NJ)),
                poff.rearrange("p (m j) -> p m j", m=3).unsqueeze(1).broadcast_to((NI, 3, 3, NJ)),
                op=ALU.mult))
            V(vector.reduce_sum(
                sg[:, 27:36], bigo.rearrange("p (m j) -> p m j", m=9),
                axis=mybir.AxisListType.X,
            ))
            assert vn[0] == VQ_ALL, vn[0]

    nc.compile()
    return nc


def host_prep(R):
    """Per-core input arrays: [96, 160] = [RjT replicated | Ri | pad]."""
    R = np.ascontiguousarray(R, np.float32)
    in_maps = []
    for core in range(NIB * NJC):
        ib, jc = divmod(core, NJC)
        rji = np.zeros((NI, 160), np.float32)
        rj = R[jc * NJ:(jc + 1) * NJ, :]              # [48, 3]
        rji[:, 0:144] = rj.T.reshape(1, 144)          # d-major, replicated
        rji[:, 144:147] = R[ib * NI:(ib + 1) * NI, :]
        in_maps.append({"rji": rji})
    return in_maps


def host_combine(partials):
    """partials: list of 8 [96,36] arrays (core order). Returns [192,18]."""
    sums = np.zeros((N, 36), np.float64)
    for core, p in enumerate(partials):
        ib = core // NJC
        sums[ib * NI:(ib + 1) * NI] += p.astype(np.float64)
    sums = sums.astype(np.float32)
    q_r = sums[:, 0:9].copy()
    q_r[:, 0] -= 1.0                                  # remove j==i self term
    s0 = q_r[:, 0:3]                                  # [N,3] n=0..2
    s1 = sums[:, 9:18].reshape(N, 3, 3)               # [N,n,d]
    s2d = sums[:, 18:27].reshape(N, 3, 3)             # [N,n,d] diagonal
    s2o = sums[:, 27:36].reshape(N, 3, 3)             # [N,n,m] off-diagonal
    ang = np.empty((N, 3, 3), np.float32)
    ang[:, :, 0] = s0 * s0
    ang[:, :, 1] = (s1 * s1).sum(-1)
    fro2 = (s2d * s2d).sum(-1) + 2.0 * (s2o * s2o).sum(-1)
    ang[:, :, 2] = 1.5 * fro2 - 0.5 * s0 * s0
    return np.concatenate([q_r, ang.reshape(N, 9)], axis=-1)


def _get_nc():
    if "nc" not in _cached:
        _cached["nc"] = build_nc()
    return _cached["nc"]


def kernel(R, box):
    R = np.asarray(R, np.float32)
    box = np.asarray(box, np.float32)
    assert R.shape == (N, 3)
    assert np.allclose(box, np.eye(3, dtype=np.float32) * BOX_L), (
        "kernel compiled for box = 20*I"
    )
    nc = _get_nc()
    in_maps = host_prep(R)
    res = run_bass_kernel_spmd(nc, in_maps, list(range(NIB * NJC)))
    partials = [res.results[c]["out"] for c in range(NIB * NJC)]
    return host_combine(partials)


# revision 16
# speedup vs baseline: 1.1239x; 1.0591x over previous
"""Trainium2 Bass kernel for the N^3 triplet descriptor (gnn_message_passing).

Strategy: the reference's O(N^3) angular sum factorizes exactly via the
Legendre addition theorem into O(N^2) per-pair vector moments:

  P0 term: (sum_j w_j)^2
  P1 term: |sum_j w_j u_j|^2                  (u = unit displacement)
  P2 term: 1.5*|sum_j w_j u_j u_j^T|_F^2 - 0.5*(sum_j w_j)^2

with w_j = fc(r_ij) * r_ij^n.  Each device accumulates 36 pair moments per
central atom (9 radial powers, 9 S1 components, 9+9 symmetric S2
components); the tiny nonlinear combine runs on host after gathering.

Sharding: 8 cores = 2 i-blocks (96 rows on partitions) x 4 j-chunks (48
neighbors on the free axis). Cross-j-chunk partials are summed on host.

Implementation: raw Bass (no Tile framework) to avoid start/tail barrier
overhead. Single ACT table (natural_log_exp_and_others): r and 1/r come
from exp(+-0.5*ln(r^2+eps)) on the otherwise-idle Scalar engine; fc is a
degree-6 polynomial in r^2 on DVE with an exact (r^2 < RC^2) cutoff mask.
"""

import numpy as np

import concourse.bass as bass
import concourse.bacc as bacc
from concourse import mybir
from concourse.bass_utils import run_bass_kernel_spmd

F32 = mybir.dt.float32
ALU = mybir.AluOpType
ACT = mybir.ActivationFunctionType

N = 192
NI = 96          # i rows per core (partition dim)
NJ = 48          # j neighbors per core (free dim)
NIB = 2          # i blocks
NJC = 4          # j chunks
BOX_L = 20.0
RC = 5.0

# fc(w) = 0.5*(1+cos(pi*sqrt(w)/RC)) as degree-6 poly in w = r^2, w in [0, RC^2]
# (chebyshev fit, max abs err 1.3e-8)
_FC_W = np.linspace(0, RC * RC, 20001)
_FC_Y = 0.5 * (1 + np.cos(np.pi * np.sqrt(_FC_W) / RC))
_FC_C = (
    np.polynomial.chebyshev.Chebyshev.fit(_FC_W, _FC_Y, 6, domain=[0, RC * RC])
    .convert(kind=np.polynomial.Polynomial)
    .coef.astype(np.float64)
)

_cached = {}


def _v(ap, off, dims):
    """Custom free-dim view of an SBUF tile AP: keep partition dim, replace
    free dims, shift offset by `off` elements."""
    return bass.AP(ap.tensor, ap.offset + off, [list(ap.ap[0])] + [list(d) for d in dims])


def build_nc():
    nc = bacc.Bacc(
        "TRN2",
        target_bir_lowering=False,
        debug=False,
        enable_asserts=True,
        num_devices=NIB * NJC,
    )
    rji_d = nc.dram_tensor("rji", [NI, 160], F32, kind="ExternalInput").ap()
    out_d = nc.dram_tensor("out", [NI, 36], F32, kind="ExternalOutput").ap()

    rji = nc.alloc_sbuf_tensor("rji_s", [NI, 160], F32).ap()
    dxr = nc.alloc_sbuf_tensor("dxr", [NI, 144], F32).ap()
    dx = nc.alloc_sbuf_tensor("dx", [NI, 144], F32).ap()
    sq_t = nc.alloc_sbuf_tensor("sq_t", [NI, 144], F32).ap()
    r2 = nc.alloc_sbuf_tensor("r2", [NI, NJ], F32).ap()
    r = nc.alloc_sbuf_tensor("r", [NI, NJ], F32).ap()
    rinv = nc.alloc_sbuf_tensor("rinv", [NI, NJ], F32).ap()
    ln1 = nc.alloc_sbuf_tensor("ln1", [NI, NJ], F32).ap()
    m25 = nc.alloc_sbuf_tensor("m25", [NI, NJ], F32).ap()
    yh = nc.alloc_sbuf_tensor("yh", [NI, NJ], F32).ap()
    yh144 = nc.alloc_sbuf_tensor("yh144", [NI, 144], F32).ap()
    r4 = nc.alloc_sbuf_tensor("r4", [NI, NJ], F32).ap()
    poff = nc.alloc_sbuf_tensor("poff", [NI, 144], F32).ap()
    fcp = nc.alloc_sbuf_tensor("fcp", [NI, 9 * NJ], F32).ap()
    w1 = nc.alloc_sbuf_tensor("w1", [NI, 144], F32).ap()
    w2 = nc.alloc_sbuf_tensor("w2", [NI, 144], F32).ap()
    tt = nc.alloc_sbuf_tensor("tt", [NI, 432], F32).ap()
    bigd = nc.alloc_sbuf_tensor("bigd", [NI, 432], F32).ap()
    bigo = nc.alloc_sbuf_tensor("bigo", [NI, 432], F32).ap()
    sg = nc.alloc_sbuf_tensor("sg", [NI, 36], F32).ap()
    scr = nc.alloc_sbuf_tensor("scr", [1, 8], F32).ap()

    dsem = nc.alloc_semaphore("dsem")
    vq = nc.alloc_semaphore("vq")      # DVE instruction counter
    sqm = nc.alloc_semaphore("sqm")    # ACT instruction counter
    gq = nc.alloc_semaphore("gq")      # GpSimd instruction counter

    rj3 = rji[:, 0:144].rearrange("p (d j) -> p d j", d=3)
    ri3 = rji[:, 144:147].unsqueeze(-1).broadcast_to((NI, 3, NJ))
    dxr3 = dxr.rearrange("p (d j) -> p d j", d=3)
    dx3 = dx.rearrange("p (d j) -> p d j", d=3)
    rinv3 = rinv.unsqueeze(1).broadcast_to((NI, 3, NJ))

    c = [float(x) for x in _FC_C]

    # DVE instruction indices (vq value after each) for cross-engine waits
    VQ_DX = 5      # dx ready
    VQ_R2 = 8      # r2 (+eps) ready
    VQ_W2 = 24     # w2 ready
    VQ_ALL = 29    # sg complete
    SQ_RINV = 2    # rinv ready
    GQ_GEO = 3     # poff + r4 ready
    GQ_BIGO = 4    # bigo ready

    with nc.Block() as block:

        @block.sync
        def _(sync):
            sync.dma_start(rji, rji_d).then_inc(dsem, 16)
            sync.wait_ge(vq, VQ_ALL)
            sync.dma_start(out_d, sg).then_inc(dsem, 16)
            sync.wait_ge(dsem, 32)

        @block.scalar
        def _(scalar):
            sn = [0]

            def S(inst):
                # same-engine ordering chain (TRN2 engines pipeline;
                # RAW hazards need explicit sems — free at runtime)
                if sn[0] > 0:
                    inst._wait_ge(sqm, sn[0])
                inst.then_inc(sqm, 1)
                sn[0] += 1
                return inst

            # dummy activation on a const tile: pulls the single ACT table
            # load (abs_reciprocal_sqrt_and_small) to t=0, overlapped with
            # the input DMA + DVE distance math
            S(scalar.activation(
                scr[0:1, 0:1], nc.const_aps.tensor(1.0, (1, 1)),
                ACT.Abs_reciprocal_sqrt))
            scalar.wait_ge(vq, VQ_R2)
            # rinv = 1/sqrt(r2 + 1e-12); r recovered on DVE as r2 * rinv
            S(scalar.activation(rinv, r2, ACT.Abs_reciprocal_sqrt))
            assert sn[0] == SQ_RINV

        @block.gpsimd
        def _(gpsimd):
            gn = [0]

            def G(inst):
                if gn[0] > 0:
                    inst._wait_ge(gq, gn[0])
                inst.then_inc(gq, 1)
                gn[0] += 1
                return inst

            # off-critical-path geometry, freeing the DVE
            gpsimd.wait_ge(vq, VQ_DX)
            G(gpsimd.tensor_tensor(
                poff[:, 0:96], dx[:, 0:96], dx[:, 48:144], op=ALU.mult))
            G(gpsimd.tensor_tensor(
                poff[:, 96:144], dx[:, 0:NJ], dx[:, 96:144], op=ALU.mult))
            gpsimd.wait_ge(vq, VQ_R2)
            G(gpsimd.tensor_tensor(r4, r2, r2, op=ALU.mult))
            assert gn[0] == GQ_GEO
            # S2 off-diag products while the DVE runs T/bigd
            gpsimd.wait_ge(vq, VQ_W2)
            G(gpsimd.tensor_tensor(
                bigo.rearrange("p (n m j) -> p n m j", n=3, m=3),
                w2.rearrange("p (n j) -> p n j", n=3).unsqueeze(2).broadcast_to((NI, 3, 3, NJ)),
                poff.rearrange("p (m j) -> p m j", m=3).unsqueeze(1).broadcast_to((NI, 3, 3, NJ)),
                op=ALU.mult))
            assert gn[0] == GQ_BIGO

        @block.vector
        def _(vector):
            vn = [0]

            def V(inst):
                if vn[0] > 0:
                    inst._wait_ge(vq, vn[0])
                inst.then_inc(vq, 1)
                vn[0] += 1
                return inst

            vector.wait_ge(dsem, 16)
            V(vector.tensor_tensor(dxr3, rj3, ri3, op=ALU.subtract))
            # minimum image (box = BOX_L * I): dx -= L*(dxr>L/2); dx += L*(dxr<-L/2)
            V(vector.tensor_scalar(
                yh144, dxr, BOX_L / 2, BOX_L, op0=ALU.is_gt, op1=ALU.mult))
            V(vector.tensor_tensor(dx, dxr, yh144, op=ALU.subtract))
            V(vector.tensor_scalar(
                yh144, dxr, -BOX_L / 2, BOX_L, op0=ALU.is_lt, op1=ALU.mult))
            V(vector.tensor_tensor(dx, dx, yh144, op=ALU.add))
            assert vn[0] == VQ_DX
            V(vector.tensor_tensor(sq_t, dx, dx, op=ALU.mult))
            V(vector.reduce_sum(
                r2, sq_t.rearrange("p (d j) -> p j d", d=3),
                axis=mybir.AxisListType.X,
            ))
            # eps keeps 1/sqrt finite at the self pair (u_ii ends up 0)
            V(vector.tensor_scalar(r2, r2, 1e-12, None, op0=ALU.add))
            assert vn[0] == VQ_R2
            # fc = poly6(r2) * (r2 < RC^2), Horner on DVE
            V(vector.tensor_scalar(m25, r2, RC * RC, None, op0=ALU.is_lt))
            V(vector.tensor_scalar(yh, r2, c[6], None, op0=ALU.mult))
            for k in (5, 4, 3, 2, 1):
                V(vector.scalar_tensor_tensor(
                    yh, yh, c[k], r2, op0=ALU.add, op1=ALU.mult))
            fc = fcp[:, 0:NJ]
            V(vector.scalar_tensor_tensor(
                fc, yh, c[0], m25, op0=ALU.add, op1=ALU.mult))
            # r = r2 * rinv (= sqrt(r2+eps)); rinv from ACT
            vector.wait_ge(sqm, SQ_RINV)
            V(vector.tensor_tensor(r, r2, rinv, op=ALU.mult))
            # fcp[k] = fc * r^k: evens via r2/r4, odds in one strided mult
            vector.wait_ge(gq, GQ_GEO)
            V(vector.tensor_tensor(
                fcp[:, 2 * NJ:3 * NJ], fc, r2, op=ALU.mult))
            V(vector.tensor_tensor(
                _v(fcp, 4 * NJ, [[2 * NJ, 2], [1, NJ]]),
                _v(fcp, 0, [[2 * NJ, 2], [1, NJ]]),
                _v(r4, 0, [[0, 2], [1, NJ]]),
                op=ALU.mult,
            ))
            V(vector.tensor_tensor(
                fcp[:, 8 * NJ:9 * NJ], fcp[:, 4 * NJ:5 * NJ], r4, op=ALU.mult))
            V(vector.tensor_tensor(
                _v(fcp, NJ, [[2 * NJ, 4], [1, NJ]]),
                _v(fcp, 0, [[2 * NJ, 4], [1, NJ]]),
                _v(r, 0, [[0, 4], [1, NJ]]),
                op=ALU.mult,
            ))
            V(vector.reduce_sum(
                sg[:, 0:9], fcp.rearrange("p (k j) -> p k j", k=9),
                axis=mybir.AxisListType.X,
            ))
            # weights: w1_n = fc r^n / r, w2_n = fc r^n / r^2
            V(vector.tensor_tensor(
                w1.rearrange("p (n j) -> p n j", n=3),
                fcp[:, 0:144].rearrange("p (n j) -> p n j", n=3),
                rinv3, op=ALU.mult))
            V(vector.tensor_tensor(
                w2.rearrange("p (n j) -> p n j", n=3),
                w1.rearrange("p (n j) -> p n j", n=3),
                rinv3, op=ALU.mult))
            # S1: T[n,d] = w1_n * dx_d
            V(vector.tensor_tensor(
                tt.rearrange("p (n d j) -> p n d j", n=3, d=3),
                w1.rearrange("p (n j) -> p n j", n=3).unsqueeze(2).broadcast_to((NI, 3, 3, NJ)),
                dx3.unsqueeze(1).broadcast_to((NI, 3, 3, NJ)),
                op=ALU.mult))
            V(vector.reduce_sum(
                sg[:, 9:18], tt.rearrange("p (m j) -> p m j", m=9),
                axis=mybir.AxisListType.X,
            ))
            # S2 diag: w2_n * dx_d^2 (sq_t);  S2 off: w2_n * (xy, yz, xz)
            V(vector.tensor_tensor(
                bigd.rearrange("p (n d j) -> p n d j", n=3, d=3),
                w2.rearrange("p (n j) -> p n j", n=3).unsqueeze(2).broadcast_to((NI, 3, 3, NJ)),
                sq_t.rearrange("p (d j) -> p d j", d=3).unsqueeze(1).broadcast_to((NI, 3, 3, NJ)),
                op=ALU.mult))
            V(vector.reduce_sum(
                sg[:, 18:27], bigd.rearrange("p (m j) -> p m j", m=9),
                axis=mybir.AxisListType.X,
            ))
            # bigo product computed on GpSimd in parallel
            vector.wait_ge(gq, GQ_BIGO)
            V(vector.reduce_sum(
                sg[:, 27:36], bigo.rearrange("p (m j) -> p m j", m=9),
                axis=mybir.AxisListType.X,
            ))
            assert vn[0] == VQ_ALL, vn[0]

    nc.compile()
    return nc


def host_prep(R):
    """Per-core input arrays: [96, 160] = [RjT replicated | Ri | pad]."""
    R = np.ascontiguousarray(R, np.float32)
    in_maps = []
    for core in range(NIB * NJC):
        ib, jc = divmod(core, NJC)
        rji = np.zeros((NI, 160), np.float32)
        rj = R[jc * NJ:(jc + 1) * NJ, :]              # [48, 3]
        rji[:, 0:144] = rj.T.reshape(1, 144)          # d-major, replicated
        rji[:, 144:147] = R[ib * NI:(ib + 1) * NI, :]
        in_maps.append({"rji": rji})
    return in_maps


def host_combine(partials):
    """partials: list of 8 [96,36] arrays (core order). Returns [192,18]."""
    sums = np.zeros((N, 36), np.float64)
    for core, p in enumerate(partials):
        ib = core // NJC
        sums[ib * NI:(ib + 1) * NI] += p.astype(np.float64)
    sums = sums.astype(np.float32)
    q_r = sums[:, 0:9].copy()
    q_r[:, 0] -= 1.0                                  # remove j==i self term
    s0 = q_r[:, 0:3]                                  # [N,3] n=0..2
    s1 = sums[:, 9:18].reshape(N, 3, 3)               # [N,n,d]
    s2d = sums[:, 18:27].reshape(N, 3, 3)             # [N,n,d] diagonal
    s2o = sums[:, 27:36].reshape(N, 3, 3)             # [N,n,m] off-diagonal
    ang = np.empty((N, 3, 3), np.float32)
    ang[:, :, 0] = s0 * s0
    ang[:, :, 1] = (s1 * s1).sum(-1)
    fro2 = (s2d * s2d).sum(-1) + 2.0 * (s2o * s2o).sum(-1)
    ang[:, :, 2] = 1.5 * fro2 - 0.5 * s0 * s0
    return np.concatenate([q_r, ang.reshape(N, 9)], axis=-1)


def _get_nc():
    if "nc" not in _cached:
        _cached["nc"] = build_nc()
    return _cached["nc"]


def kernel(R, box):
    R = np.asarray(R, np.float32)
    box = np.asarray(box, np.float32)
    assert R.shape == (N, 3)
    assert np.allclose(box, np.eye(3, dtype=np.float32) * BOX_L), (
        "kernel compiled for box = 20*I"
    )
    nc = _get_nc()
    in_maps = host_prep(R)
    res = run_bass_kernel_spmd(nc, in_maps, list(range(NIB * NJC)))
    partials = [res.results[c]["out"] for c in range(NIB * NJC)]
    return host_combine(partials)
